# revision 32
# baseline (speedup 1.0000x reference)
"""Self-contained Trainium2 kernel for nn_Attention_80436147519543.

ViTDet-style decomposed-rel-pos attention: B=8, H=W=32, C=768, 12 heads.
Sharding: data-parallel over batch across the 8 NeuronCores (1 element per
core), weights replicated, no collectives.

Per-core dataflow (matmul operands bf16, PSUM accumulation f32):
  - x is staged host-side as xT bf16 (768, 1024) per core.
  - QKV matmuls produce, per head, a combined 128-row "qcomb" rhs tile and
    "kmask" lhsT tile for a SINGLE merged K=128 score matmul that computes
    q.k/sqrt(dh) AND the decomposed rel-pos bias in one accumulation:
      even heads: rows 0-63 = head dims, 64-95 = rel_hT, 96-127 = rel_wT
      odd heads:  rows 0-63 = rel_hT/rel_wT, 64-127 = head dims
    (the parity split falls out of the QKV psum halves for free). kmask
    carries k^T/sqrt(dh) in the dim rows and a constant 0/1 mask in the
    other 64 rows that selects rel_hT[ki] + rel_wT[kj] per key partition.
  - rel_hT/rel_wT come from G = rel_table_rev.T @ q (one matmul per query
    chunk) followed by a diagonal-band gather PSUM -> SBUF bf16 -> DRAM ->
    SBUF via a negative-stride 3D access pattern; rel_w's G matmul reads
    queries in digit-swapped order so its gather stays contiguous, then one
    strided DVE copy un-permutes.
  - scores are TRANSPOSED (keys on partitions, queries free), exp on
    ScalarE over (128,1024) tiles (scores bounded, no max subtraction).
    The attention phase is ACT(exp)-rate-limited, so QKV/rel prep for head
    pairs 2-5 is software-pipelined INTO the attention loop as PE filler
    work (2 matmuls per key-tile iteration) to hide PE idle time.
  - O^T = v_aug.T @ exp(S^T) accumulated over key tiles; the appended ones
    column of v gives the softmax denominator in row 64. Normalization
    multiplies O^T rows by the replicated reciprocal (commutes with the
    output projection).
  - Y = O_all @ w_out + b_out in natural (token, feature) layout.
"""

from collections import deque as _deque
from dataclasses import replace as _ap_replace

import numpy as np

NUM_HEADS = 12
B, H, W, C = 8, 32, 32, 768
DH = C // NUM_HEADS          # 64
HW = H * W                   # 1024
NPAIR = NUM_HEADS // 2       # 6
QC = 2                       # query chunks of 512
KT = 8                       # key tiles of 128
_COMPILED = None


def _build():
    import concourse.mybir as mybir
    from concourse import bacc
    from concourse.tile import TileContext

    f32 = mybir.dt.float32
    bf16 = mybir.dt.bfloat16
    Act = mybir.ActivationFunctionType
    Alu = mybir.AluOpType

    nc = bacc.Bacc("TRN2", target_bir_lowering=False, debug=False, num_devices=8)

    xT_d = nc.dram_tensor("xT", [C, HW], bf16, kind="ExternalInput").ap()
    wq_d = nc.dram_tensor("wqkv", [C, 3 * C], bf16, kind="ExternalInput").ap()
    wo_d = nc.dram_tensor("wout", [C, C], bf16, kind="ExternalInput").ap()
    bqk_d = nc.dram_tensor("bqk", [2 * C, 1], f32, kind="ExternalInput").ap()
    bvb_d = nc.dram_tensor("bvb", [128, C], f32, kind="ExternalInput").ap()
    bob_d = nc.dram_tensor("bob", [128, C], f32, kind="ExternalInput").ap()
    relT_d = nc.dram_tensor("relT", [128, 126], bf16, kind="ExternalInput").ap()
    mask_d = nc.dram_tensor("mask", [64, HW], bf16, kind="ExternalInput").ap()
    y_d = nc.dram_tensor("y", [HW, C], f32, kind="ExternalOutput").ap()

    with TileContext(nc) as tc:
        with (
            tc.tile_pool(name="const", bufs=1) as cp,
            tc.tile_pool(name="work", bufs=2) as wp,
            tc.tile_pool(name="dram", bufs=3, space="DRAM") as dp,
            tc.tile_pool(name="ps_mm", bufs=2, space="PSUM") as ps_mm,
            tc.tile_pool(name="ps_ot", bufs=2, space="PSUM") as ps_ot,
        ):
            # ---- constant loads: tiny critical tensors first -------------
            relT_sb = cp.tile([128, 126], bf16)
            nc.sync.dma_start(relT_sb[:], relT_d[:])
            bqk_sb = cp.tile([128, 12], f32)
            nc.sync.dma_start(
                bqk_sb[:],
                bqk_d.rearrange("(ft p) one -> p (ft one)", p=128),
            )

            # xT + the qk columns of wqkv feed the critical path; v columns
            # and w_out follow (separate tensors so qk matmuls don't pick up
            # a false dep on the later v-column loads).
            wq_sb, xT_sb = [], []
            for ct in range(6):
                tx = cp.tile([128, HW], bf16, name=f"xT{ct}")
                nc.sync.dma_start(tx[:], xT_d[128 * ct:128 * (ct + 1), :])
                xT_sb.append(tx)
                tw = cp.tile([128, 2 * C], bf16, name=f"wq{ct}")
                nc.sync.dma_start(tw[:], wq_d[128 * ct:128 * (ct + 1), 0:2 * C])
                wq_sb.append(tw)
            wv_sb = []
            for ct in range(6):
                t = cp.tile([128, C], bf16, name=f"wv{ct}")
                nc.sync.dma_start(t[:], wq_d[128 * ct:128 * (ct + 1), 2 * C:])
                wv_sb.append(t)
            wo_sb = []
            for ct in range(6):
                t = cp.tile([128, C], bf16, name=f"wo{ct}")
                nc.sync.dma_start(t[:], wo_d[128 * ct:128 * (ct + 1), :])
                wo_sb.append(t)
            bvb_sb = cp.tile([128, C], f32)
            nc.scalar.dma_start(bvb_sb[:], bvb_d[:])
            bob_sb = cp.tile([128, C], f32)
            nc.scalar.dma_start(bob_sb[:], bob_d[:])

            qcomb = [cp.tile([128, HW], bf16, name=f"qc{h}")
                     for h in range(NUM_HEADS)]
            kmask = [cp.tile([128, HW], bf16, name=f"km{h}")
                     for h in range(NUM_HEADS)]

            GATHER_AP = [[1024, 32], [-992, 32], [1, 32]]

            def emit_qk(ft):
                """One q or k feature tile (head pair): 12 MMs, 4 epilogues.
                Yields after each PE matmul."""
                isq = ft < 6
                pair = ft if isq else ft - 6
                dst = qcomb if isq else kmask
                scale = 1.0 if isq else 0.125
                for qc in range(QC):
                    qs = slice(512 * qc, 512 * (qc + 1))
                    ps = ps_mm.tile([128, 512], f32, tag="qk", name="ps_qk")
                    for ct in range(6):
                        nc.tensor.matmul(
                            ps[:],
                            wq_sb[ct][:, 128 * ft:128 * (ft + 1)],
                            xT_sb[ct][:, qs],
                            start=(ct == 0), stop=(ct == 5),
                        )
                        yield
                    nc.vector.tensor_scalar(
                        dst[2 * pair][0:64, qs], ps[0:64, :],
                        bqk_sb[0:64, ft:ft + 1], scale, Alu.add, Alu.mult,
                    )
                    nc.vector.tensor_scalar(
                        dst[2 * pair + 1][64:128, qs], ps[64:128, :],
                        bqk_sb[64:128, ft:ft + 1], scale, Alu.add, Alu.mult,
                    )

            def emit_rel(h, g_on_act=False):
                """rel_hT/rel_wT rows of qcomb[h]. Yields after each PE MM."""
                gcopy = nc.scalar.copy if g_on_act else nc.vector.tensor_copy
                ro = 64 * (h % 2)
                go = 64 * (1 - h % 2)
                gsb = wp.tile([128, HW], bf16, tag="gsb", bufs=3, name="gsb")
                nc.gpsimd.memset(gsb[:], 0.0)
                for qc in range(QC):
                    qs = slice(512 * qc, 512 * (qc + 1))
                    ps = ps_mm.tile([63, 512], f32, tag="qk", name="ps_gh")
                    nc.tensor.matmul(
                        ps[:], relT_sb[ro:ro + 64, 0:63],
                        qcomb[h][ro:ro + 64, qs],
                        start=True, stop=True, tile_position=(ro, 0),
                    )
                    yield
                    gcopy(gsb[0:63, qs], ps[:])
                qperm = qcomb[h][ro:ro + 64, :].rearrange(
                    "p (qo qi) -> p qi qo", qo=32)
                for pc in range(QC):
                    ps = ps_mm.tile([63, 512], f32, tag="qk", name="ps_gw")
                    nc.tensor.matmul(
                        ps[:], relT_sb[ro:ro + 64, 63:126],
                        qperm[:, 16 * pc:16 * (pc + 1), :],
                        start=True, stop=True, tile_position=(ro, 0),
                    )
                    yield
                    gcopy(gsb[64:127, 512 * pc:512 * (pc + 1)], ps[:])
                gd = dp.tile([128, HW], bf16, tag="gd", name="gd")
                nc.scalar.dma_start(gd[:], gsb[:])
                src_h = _ap_replace(gd, offset=gd.offset + 31 * HW, ap=GATHER_AP)
                nc.sync.dma_start(
                    qcomb[h][go:go + 32, :].rearrange("p (i m) -> p i m", i=32),
                    src_h,
                )
                stg = wp.tile([32, HW], bf16, tag="stg", bufs=3, name="stg")
                src_w = _ap_replace(gd, offset=gd.offset + (64 + 31) * HW,
                                    ap=GATHER_AP)
                nc.sync.dma_start(
                    stg.rearrange("p (qi qo) -> p qi qo", qi=32), src_w,
                )
                nc.vector.tensor_copy(
                    qcomb[h][go + 32:go + 64, :].rearrange(
                        "p (qo qi) -> p qi qo", qo=32),
                    stg.rearrange("p (qi qo) -> p qi qo", qi=32),
                )

            def emit_pair(p, g_on_act=False):
                yield from emit_qk(p)        # q features
                yield from emit_qk(6 + p)    # k features
                yield from emit_rel(2 * p, g_on_act)
                yield from emit_rel(2 * p + 1, g_on_act)

            def run_all(gen):
                for _ in gen:
                    pass

            # mask rows for the merged score matmuls (needed from the
            # first attention head onward, well after the input loads)
            for h in range(NUM_HEADS):
                mo = 64 * (1 - h % 2)
                nc.scalar.dma_start(kmask[h][mo:mo + 64, :], mask_d[:])

            # ---- prep pairs 0-1 up front ---------------------------------
            run_all(emit_pair(0, g_on_act=True))
            run_all(emit_pair(1, g_on_act=True))
            # ---- v projection (natural layout + ones col per head) -------
            vaug_sb = [cp.tile([128, 65 * NUM_HEADS], bf16, name=f"va{tt}")
                       for tt in range(8)]
            def emit_v(tt):
                nc.gpsimd.memset(vaug_sb[tt][:], 1.0)
                for fc, w in ((0, 512), (512, 256)):
                    ps = ps_mm.tile([128, 512], f32, tag="qk", name="ps_v")
                    for ct in range(6):
                        nc.tensor.matmul(
                            ps[:, :w],
                            xT_sb[ct][:, 128 * tt:128 * (tt + 1)],
                            wv_sb[ct][:, fc:fc + w],
                            start=(ct == 0), stop=(ct == 5),
                        )
                        yield
                    vtmp = wp.tile([128, 512], bf16, tag="vtmp", bufs=4, name="vtmp")
                    nc.vector.tensor_tensor(
                        vtmp[:, :w], ps[:, :w], bvb_sb[:, fc:fc + w], Alu.add)
                    h0, nh = fc // 64, w // 64
                    nc.scalar.dma_start(
                        vaug_sb[tt][:, 65 * h0:65 * (h0 + nh)].rearrange(
                            "p (h m) -> p h m", m=65)[:, :, 0:64],
                        vtmp[:, :w].rearrange("p (h m) -> p h m", m=64),
                    )

            for tt in range(5):
                run_all(emit_v(tt))

            # proj pass 1: heads 0-9 contribution (OallT tiles 0-4),
            # + b_out; runs as late-attention filler, completing into ypart.
            ypart_sb = [cp.tile([128, C], f32, name=f"yp{tt}")
                        for tt in range(8)]

            def emit_proj1():
                for tt in range(8):
                    for fc, w in ((0, 512), (512, 256)):
                        ps = ps_mm.tile([128, 512], f32, tag="qk",
                                        name="ps_y1")
                        for ct in range(5):
                            while ct >= completed_pairs[0]:
                                yield True   # not ready -- tells consumer to stop
                            nc.tensor.matmul(
                                ps[:, :w],
                                OallT_sb[ct][:, 128 * tt:128 * (tt + 1)],
                                wo_sb[ct][:, fc:fc + w],
                                start=(ct == 0), stop=(ct == 4),
                            )
                            yield
                        nc.vector.tensor_tensor(
                            ypart_sb[tt][:, fc:fc + w], ps[:, :w],
                            bob_sb[:, fc:fc + w], Alu.add,
                        )

            # v tail, pairs 2-5, then proj pass 1 feed the attention loop
            v_emitted = [5]          # vaug tiles fully emitted so far
            completed_pairs = [0]    # OallT tiles whose norms are emitted

            def fillers():
                for tt in range(5, 8):
                    yield from emit_v(tt)
                    v_emitted[0] = tt + 1
                for p in range(2, NPAIR):
                    yield from emit_pair(p)
                yield from emit_proj1()
            filler = fillers()
            filler_done = [False]

            def consume_filler(n):
                """Advance up to n filler units; stops early if the filler
                reports its next producer hasn't been emitted yet."""
                if filler_done[0]:
                    return
                try:
                    for _ in range(n):
                        if next(filler):
                            break
                except StopIteration:
                    filler_done[0] = True

            # ---- attention per head --------------------------------------
            OallT_sb = [cp.tile([128, HW], bf16, name=f"oT{p}")
                        for p in range(NPAIR)]

            def emit_pv(ot, wt, h, kt):
                for qc in range(QC):
                    nc.tensor.matmul(
                        ot[qc][:], vaug_sb[kt][:, 65 * h:65 * (h + 1)],
                        wt[:, 512 * qc:512 * (qc + 1)],
                        start=(kt == 0), stop=(kt == KT - 1),
                    )

            def emit_norm(h, ot):
                for qc in range(QC):
                    qs = slice(512 * qc, 512 * (qc + 1))
                    rr = wp.tile([1, 512], f32, tag="rr", name="rr")
                    nc.vector.reciprocal(rr[:], ot[qc][64:65, :])
                    rrep = wp.tile([64, 512], f32, tag="rrep", name="rrep")
                    nc.gpsimd.partition_broadcast(rrep[:], rr[:])
                    nc.vector.tensor_tensor(
                        OallT_sb[h // 2][64 * (h % 2):64 * (h % 2) + 64, qs],
                        ot[qc][0:64, :], rrep[:], Alu.mult,
                    )

            # PVs trail their exp by ~3 key-tiles so PE never parks on
            # ACT; a PV only flushes once its vaug tile has been emitted
            # (Tile deps follow emission order), and norms ride behind the
            # last PV of each head.
            pending = _deque()

            def flush_pv():
                fh, fot, fwt, fkt = pending.popleft()
                emit_pv(fot, fwt, fh, fkt)
                if fkt == KT - 1:
                    emit_norm(fh, fot)
                    completed_pairs[0] = (fh + 1) // 2

            slot = 0
            for h in range(NUM_HEADS):
                ot = [ps_ot.tile([65, 512], f32, tag="ot", name=f"ot{qc}")
                      for qc in range(QC)]
                for kt in range(KT):
                    st = ps_mm.tile([128, HW], f32, tag="st", name="st")
                    for qc in range(QC):
                        qs = slice(512 * qc, 512 * (qc + 1))
                        nc.tensor.matmul(
                            st[:, qs], kmask[h][:, 128 * kt:128 * (kt + 1)],
                            qcomb[h][:, qs], start=True, stop=True,
                        )
                    wt = wp.tile([128, HW], bf16, tag="wt", bufs=6, name="wt")
                    nc.scalar.activation(wt[:], st[:], Act.Exp)
                    pending.append((h, ot, wt, kt))
                    while len(pending) > 3 and v_emitted[0] > pending[0][3]:
                        flush_pv()
                    while len(pending) > 5:
                        # wt buffers run out before vaug is emitted: force
                        # the v fillers forward.
                        consume_filler(1)
                        while len(pending) > 3 and v_emitted[0] > pending[0][3]:
                            flush_pv()
                    consume_filler(4 if slot < 12 else 2)
                    slot += 1
            while pending:
                while not (v_emitted[0] > pending[0][3]):
                    consume_filler(1)
                flush_pv()

            consume_filler(1 << 30)   # drain pairs/proj1 leftovers

            # ---- output projection (pass 2: heads 8-11 contribution) -----
            for tt in range(8):
                ysb = wp.tile([128, C], f32, tag="y", name="ysb")
                for fc, w in ((0, 512), (512, 256)):
                    ps = ps_mm.tile([128, 512], f32, tag="qk", name="ps_y")
                    nc.tensor.matmul(
                        ps[:, :w],
                        OallT_sb[5][:, 128 * tt:128 * (tt + 1)],
                        wo_sb[5][:, fc:fc + w],
                        start=True, stop=True,
                    )
                    nc.vector.tensor_tensor(
                        ysb[:, fc:fc + w], ps[:, :w],
                        ypart_sb[tt][:, fc:fc + w], Alu.add,
                    )
                nc.scalar.dma_start(y_d[128 * tt:128 * (tt + 1), :], ysb[:])

    nc.compile()
    return nc


def _get_compiled():
    global _COMPILED
    if _COMPILED is None:
        _COMPILED = _build()
    return _COMPILED


def _host_inputs(x, w_qkv, b_qkv, w_out, b_out, rel_pos_h, rel_pos_w):
    import ml_dtypes
    bf16 = ml_dtypes.bfloat16

    x = np.asarray(x, np.float32).reshape(B, HW, C)
    w_qkv = np.asarray(w_qkv, np.float32)
    b_qkv = np.asarray(b_qkv, np.float32)
    w_out = np.asarray(w_out, np.float32)
    b_out = np.asarray(b_out, np.float32)
    rel_pos_h = np.asarray(rel_pos_h, np.float32)
    rel_pos_w = np.asarray(rel_pos_w, np.float32)

    wq_bf = np.ascontiguousarray(w_qkv).astype(bf16)
    wo_bf = np.ascontiguousarray(w_out).astype(bf16)

    bqk = np.concatenate([b_qkv[:C], b_qkv[C:2 * C]]).reshape(2 * C, 1)
    bvb = np.broadcast_to(b_qkv[2 * C:], (128, C)).astype(np.float32).copy()
    bob = np.broadcast_to(b_out, (128, C)).astype(np.float32).copy()

    relT = np.zeros((128, 126), np.float32)
    relT[:64, 0:63] = rel_pos_h[::-1].T
    relT[:64, 63:126] = rel_pos_w[::-1].T
    relT[64:] = relT[:64]

    mask = np.zeros((64, HW), np.float32)
    pp = np.arange(128)
    for t in range(8):
        blk = np.zeros((64, 128), np.float32)
        blk[:32][(4 * t + pp // 32), pp] = 1.0
        blk[32:][(pp % 32), pp] = 1.0
        mask[:, 128 * t:128 * (t + 1)] = blk

    const = {
        "wqkv": wq_bf, "wout": wo_bf,
        "bqk": bqk.astype(np.float32), "bvb": bvb, "bob": bob,
        "relT": relT.astype(bf16), "mask": mask.astype(bf16),
    }
    return [
        dict(const, xT=np.ascontiguousarray(x[b].T).astype(bf16))
        for b in range(B)
    ]


def kernel(x, w_qkv, b_qkv, w_out, b_out, rel_pos_h, rel_pos_w, _trace=False):
    from concourse import bass_utils

    nc = _get_compiled()
    in_maps = _host_inputs(x, w_qkv, b_qkv, w_out, b_out, rel_pos_h, rel_pos_w)
    res = bass_utils.run_bass_kernel_spmd(
        nc, in_maps, core_ids=list(range(8)), trace=_trace,
    )
    y = np.stack([np.asarray(res.results[b]["y"], np.float32) for b in range(B)])
    out = y.reshape(B, H, W, C)
    if _trace:
        return out, res
    return out
